# revision 20
# baseline (speedup 1.0000x reference)
"""Multi-head attention (B=2, S=2048, D=1024, H=16) on 8 NeuronCores.

Sharding: 2-way batch x 4-way heads (4 heads / core). Each core computes
its 4 heads' attention output projected through its slice of Wo, giving a
partial [S, D] output; the host sums the 4 partials per batch element and
adds the bias terms (bo and the softmax-folded bv @ Wo.T).

Device layout notes:
  - Host pre-transposes activations (x.T, D-major) so every matmul
    contraction dim lands on SBUF partitions without on-device transposes
    of the big activations.
  - Projections produce Q.T/K.T/V.T [256, 2048] (feature-major), which is
    exactly the layout the scores matmul wants. V.T is flipped to
    token-major V via small PE transposes (64 x [64,128]).
  - Scores are computed transposed (S.T = K Q.T chunks) so the exp'd
    attention tiles feed A@V as the moving operand with V as stationary.
  - softmax skips max-subtraction (scores ~ N(0,1), max << fp32 range) and
    row sums come free via a ones-column appended to V (row 64 of the
    O.T psum accumulator); normalization is folded in after A@V.
  - Matmul operands are bitcast to float32r: 1 PE cycle/row at N>=512
    (plain fp32 is 4 cycles/row).
"""

import numpy as np

import concourse.bass as bass
import concourse.mybir as mybir
import concourse.tile as tile
from concourse.bass_utils import run_bass_kernel_spmd
from concourse.masks import make_identity

D_MODEL = 1024
S = 2048
B = 2
H = 4            # heads per core
DK = 64
F = H * DK       # 256 local features per core
KD = D_MODEL // 128   # 8 contraction chunks for projections
TK = S // 128         # 16 token chunks
NQ = S // 512         # 4 moving-dim slices

f32 = mybir.dt.float32
f32r = mybir.dt.float32r


def _dep_nop(nc, engine, *producers):
    """Emit PE nops depending on the given producer instructions.

    The fused weight-load of a float32r matmul can carry only one sync
    wait in its ISA struct; a matmul whose dependencies span several
    semaphores fails walrus codegen ("Too many sync wait commands").
    A nop per producer absorbs those waits on the PE queue so the
    following matmuls need at most one fresh wait each.
    """
    from concourse.tile import add_dep_helper
    for p in producers:
        if p is None:
            continue
        nop = engine.nop(hint="dep")
        add_dep_helper(nop.ins, p.ins, reason="absorb wait")


def _pe_dep_nop(nc, *producers):
    _dep_nop(nc, nc.tensor, *producers)


def _fix_matmul_waits(nc):
    """Peel extra sync waits off float32r matmuls onto PE NoOps.

    Walrus places a 4-byte-dtype matmul's waits in its fused weight-load
    ISA struct, which fits only one wait; more fail codegen with "Too
    many sync wait commands". Moving all but one wait onto NoOps
    inserted immediately before the matmul on the same engine keeps
    identical ordering semantics.
    """
    import bass_rust
    n = 0
    capped = tuple(
        t for t in (getattr(mybir, n, None) for n in (
            "InstMatmult", "InstDMACopy", "InstActivation",
            "InstTensorCopy", "InstTensorTensor", "InstReciprocal",
            "InstMemset", "InstTensorScalarAffineSelect",
            "InstTensorScalarPtr", "InstTensorScalar",
            "InstTensorReduce", "InstCopy", "InstDrain",
            "InstEventSemaphore", "InstNoOp"))
        if isinstance(t, type))

    for f in nc.m.functions:
        for blk in f.blocks:
            newlist = []
            for ins in blk.instructions:
                si = ins.sync_info
                if (isinstance(ins, capped) and si is not None
                        and si.on_wait and len(si.on_wait) > 1):
                    waits = list(si.on_wait)
                    for w in waits[:-1]:
                        nop = mybir.InstNoOp(name=f"I-wfix{n}", ins=[], outs=[])
                        n += 1
                        nop.engine = ins.engine
                        nop.sync_info = bass_rust.SyncInfo(
                            on_wait=[w], on_update=[])
                        newlist.append(nop)
                    ins.sync_info = bass_rust.SyncInfo(
                        on_wait=[waits[-1]], on_update=list(si.on_update))
                newlist.append(ins)
            blk.instructions = newlist
    return n


def build_nc():
    nc = bass.Bass("TRN2", target_bir_lowering=False, debug=False)

    xqT = nc.dram_tensor("xqT", [D_MODEL, S], f32r, kind="ExternalInput").ap()
    xkT = nc.dram_tensor("xkT", [D_MODEL, S], f32r, kind="ExternalInput").ap()
    xvT = nc.dram_tensor("xvT", [D_MODEL, S], f32r, kind="ExternalInput").ap()
    wqT = nc.dram_tensor("wqT", [D_MODEL, F], f32r, kind="ExternalInput").ap()
    wkT = nc.dram_tensor("wkT", [D_MODEL, F], f32r, kind="ExternalInput").ap()
    wvT = nc.dram_tensor("wvT", [D_MODEL, F], f32r, kind="ExternalInput").ap()
    woT = nc.dram_tensor("woT", [F, D_MODEL], f32r, kind="ExternalInput").ap()
    bq8 = nc.dram_tensor("bq8", [F], f32, kind="ExternalInput").ap()
    bk_ = nc.dram_tensor("bk_", [F], f32, kind="ExternalInput").ap()
    out = nc.dram_tensor("out", [S, D_MODEL], f32, kind="ExternalOutput").ap()

    with tile.TileContext(nc) as tc:
        with (
            tc.tile_pool(name="wpool", bufs=1) as wpool,
            tc.tile_pool(name="qkpool", bufs=1) as qkpool,
            tc.tile_pool(name="vpool", bufs=1) as vpool,
            tc.tile_pool(name="otpool", bufs=1) as otpool,
        ):
            # ---- persistent SBUF: weights, biases, identity ----
            wq_sb = wpool.tile([128, KD, F], f32r, tag="wq")
            wk_sb = wpool.tile([128, KD, F], f32r, tag="wk")
            wv_sb = wpool.tile([128, KD, F], f32r, tag="wv")
            wo_sb = wpool.tile([64, H, D_MODEL], f32r, tag="wo")
            bq_sb = wpool.tile([128, 2], f32, tag="bq")
            bk_sb = wpool.tile([128, 2], f32, tag="bk")
            id_sb = wpool.tile([128, 64], f32, tag="id")
            ones65 = wpool.tile([65, 64], f32r, tag="ones65")
            ones_f = wpool.tile([128, 64], f32, tag="ones_f")

            wdma = {}
            wdma["q"] = nc.sync.dma_start(wq_sb, wqT.rearrange("(c p) f -> p c f", p=128))
            wdma["k"] = nc.sync.dma_start(wk_sb, wkT.rearrange("(c p) f -> p c f", p=128))
            wdma["v"] = nc.sync.dma_start(wv_sb, wvT.rearrange("(c p) f -> p c f", p=128))
            wdma["o"] = nc.sync.dma_start(wo_sb, woT.rearrange("(h p) i -> p h i", p=64))
            nc.gpsimd.dma_start(bq_sb, bq8.rearrange("(f p) -> p f", p=128))
            nc.gpsimd.dma_start(bk_sb, bk_.rearrange("(f p) -> p f", p=128))
            # identity blocks at partition bases 0 and 64 (make_identity,
            # inlined to capture the instructions for wait absorption)
            nc.gpsimd.memset(id_sb, 0.0)
            # memset cannot emit float32r; stage in f32 and copy (rounds)
            nc.vector.memset(ones_f, 1.0)
            nc.vector.tensor_copy(ones65, ones_f[0:65, :])
            id_insts = []
            for r0 in (0, 64):
                id_insts.append(nc.gpsimd.affine_select(
                    out=id_sb[r0:r0 + 64, :], in_=id_sb[r0:r0 + 64, :],
                    compare_op=mybir.AluOpType.not_equal, fill=1.0, base=0,
                    pattern=[[-1, 64]], channel_multiplier=1,
                ))

            # persistent activations
            qt_sb = qkpool.tile([128, 2, S], f32r, tag="qt")   # Q.T/8 (+bq/8)
            kt_sb = qkpool.tile([128, 2, S], f32r, tag="kt")   # K.T (+bk)
            vaug = vpool.tile([128, TK, H, 65], f32r, tag="vaug")  # V tok-major + ones
            otn = otpool.tile([64, H, S], f32r, tag="otn")     # normalized O.T per head

            ones_inst = nc.vector.tensor_copy(
                vaug[:, :, :, 64:65],
                ones_f.rearrange("p (a b c) -> p a b c", a=TK, b=H))

            # ---- Phase A: projections Q.T, K.T, V.T ----
            with (
                tc.tile_pool(name="xpool", bufs=3) as xpool,
                tc.tile_pool(name="vtpool", bufs=1) as vtpool,
            ):
                vt_sb = vtpool.tile([128, 2, S], f32, tag="vt")
                evac_insts = []   # previous projection's psum-evacuating ACT ops
                vt_evacs = []
                with tc.tile_pool(name="psA", bufs=2, space="PSUM") as psA:
                    for proj, (xT, w_sb) in enumerate(
                        ((xqT, wq_sb), (xkT, wk_sb), (xvT, wv_sb))
                    ):
                        ps = [
                            psA.tile([128, S], f32, name=f"ps{proj}_{f}", tag="proj")
                            for f in range(2)
                        ]
                        _pe_dep_nop(nc, wdma["qkv"[proj]], *evac_insts)
                        evac_insts = []
                        for kc in range(KD):
                            xc = xpool.tile([128, S], f32r, tag="xc")
                            nc.sync.dma_start(xc, xT[kc * 128:(kc + 1) * 128, :])
                            for f in range(2):
                                lhsT = w_sb[:, kc, f * 128:(f + 1) * 128]
                                for qn in range(NQ):
                                    nc.tensor.matmul(
                                        ps[f][:, qn * 512:(qn + 1) * 512],
                                        lhsT,
                                        xc[:, qn * 512:(qn + 1) * 512],
                                        start=(kc == 0),
                                        stop=(kc == KD - 1),
                                    )
                        for f in range(2):
                            if proj == 0:   # Q: (Q + bq)/8
                                evac_insts.append(nc.scalar.activation(
                                    qt_sb[:, f, :], ps[f],
                                    mybir.ActivationFunctionType.Identity,
                                    bias=bq_sb[:, f:f + 1], scale=0.125,
                                ))
                            elif proj == 1:  # K: K + bk
                                evac_insts.append(nc.scalar.activation(
                                    kt_sb[:, f, :], ps[f],
                                    mybir.ActivationFunctionType.Identity,
                                    bias=bk_sb[:, f:f + 1], scale=1.0,
                                ))
                            else:            # V: plain copy
                                vt_evacs.append(nc.scalar.copy(vt_sb[:, f, :], ps[f]))

                # V.T -> token-major V via PE transposes
                with tc.tile_pool(name="psTr", bufs=4, space="PSUM") as psTr:
                    _pe_dep_nop(nc, *id_insts, *evac_insts)
                    for h in range(H):
                        fc, r0 = h // 2, 64 * (h % 2)
                        for tcn in range(TK):
                            ptr = psTr.tile([128, 64], f32, tag="tr")
                            nc.tensor.transpose(
                                ptr,
                                vt_sb[r0:r0 + 64, fc, tcn * 128:(tcn + 1) * 128],
                                id_sb[r0:r0 + 64, :],
                            )
                            nc.scalar.copy(vaug[:, tcn, h, 0:64], ptr)

            # ---- Phase B/C: per-head attention ----
            with (
                tc.tile_pool(name="atpool", bufs=3) as atpool,
                tc.tile_pool(name="rpool", bufs=2) as rpool,
                tc.tile_pool(name="psS", bufs=2, space="PSUM") as psS,
                tc.tile_pool(name="psO", bufs=1, space="PSUM") as psO,
            ):
                norm_insts = [ones_inst]
                last_mul = None
                for h in range(H):
                    fc, r0 = h // 2, 64 * (h % 2)
                    qt_h = qt_sb[r0:r0 + 64, fc, :]
                    kt_h = kt_sb[r0:r0 + 64, fc, :]
                    ot_ps = psO.tile([65, S], f32, tag="ot")
                    _pe_dep_nop(nc, *norm_insts)
                    norm_insts = []
                    for kc in range(TK):
                        at = atpool.tile([128, S], f32r, tag="at")
                        lhsT = kt_h[:, kc * 128:(kc + 1) * 128]
                        for qh in range(2):
                            stp = psS.tile([128, 1024], f32, tag="st")
                            for qn in range(2):
                                q0 = qh * 1024 + qn * 512
                                nc.tensor.matmul(
                                    stp[:, qn * 512:(qn + 1) * 512],
                                    lhsT,
                                    qt_h[:, q0:q0 + 512],
                                    start=True, stop=True,
                                )
                            nc.scalar.activation(
                                at[:, qh * 1024:(qh + 1) * 1024], stp,
                                mybir.ActivationFunctionType.Exp,
                            )
                        for qn in range(NQ):
                            nc.tensor.matmul(
                                ot_ps[:, qn * 512:(qn + 1) * 512],
                                vaug[:, kc, h, :],
                                at[:, qn * 512:(qn + 1) * 512],
                                start=(kc == 0),
                                stop=(kc == TK - 1),
                            )
                    # normalization: otn = ot / rowsum (rowsum = psum row
                    # 64). The reciprocal row is broadcast across partitions
                    # with a K=1 PE outer product against a ones column.
                    rec = rpool.tile([65, S], f32r, tag="rec")
                    with nc.allow_low_precision(
                            reason="f32r is f32 bits; matmul needs f32r"):
                        nc.vector.reciprocal(rec[64:65, :], ot_ps[64:65, :])
                    for qn in range(NQ):
                        q0 = qn * 512
                        pbc = psS.tile([64, 512], f32,
                                       name=f"pbc{h}_{qn}", tag="st")
                        nc.tensor.matmul(
                            pbc, ones65[64:65, :], rec[64:65, q0:q0 + 512],
                            start=True, stop=True,
                        )
                        bcs = rpool.tile([64, 512], f32, tag="bcs")
                        nc.vector.tensor_copy(bcs, pbc)
                        last_mul = nc.vector.tensor_mul(
                            otn[:, h, q0:q0 + 512],
                            ot_ps[0:64, q0:q0 + 512], bcs)
                    norm_insts = [last_mul]

            # ---- Phase D: output projection partial = O_cat @ Wo_g.T ----
            with (
                tc.tile_pool(name="outpool", bufs=3) as outpool,
                tc.tile_pool(name="psD", bufs=4, space="PSUM") as psD,
            ):
                _pe_dep_nop(nc, wdma["o"], last_mul)
                for tcn in range(TK):
                    pd = psD.tile([128, D_MODEL], f32, tag="pd")
                    for h in range(H):
                        for n2 in range(2):
                            nc.tensor.matmul(
                                pd[:, n2 * 512:(n2 + 1) * 512],
                                otn[:, h, tcn * 128:(tcn + 1) * 128],
                                wo_sb[:, h, n2 * 512:(n2 + 1) * 512],
                                start=(h == 0),
                                stop=(h == H - 1),
                            )
                    ob = outpool.tile([128, D_MODEL], f32, tag="ob")
                    nc.vector.tensor_copy(ob, pd)
                    nc.sync.dma_start(out[tcn * 128:(tcn + 1) * 128, :], ob)

    _fix_matmul_waits(nc)
    return nc


_NC = None


def _get_nc():
    global _NC
    if _NC is None:
        _NC = build_nc()
    return _NC


def make_in_maps(q, k, v, Wq, bq, Wk, bk, Wv, bv, Wo, bo):
    q = np.asarray(q, np.float32)
    k = np.asarray(k, np.float32)
    v = np.asarray(v, np.float32)
    xT = {}
    for b in range(B):
        xT[("q", b)] = np.ascontiguousarray(q[b].T)
        xT[("k", b)] = np.ascontiguousarray(k[b].T)
        xT[("v", b)] = np.ascontiguousarray(v[b].T)
    in_maps = []
    for c in range(8):
        b, g = divmod(c, 4)
        sl = slice(F * g, F * (g + 1))
        in_maps.append({
            "xqT": xT[("q", b)],
            "xkT": xT[("k", b)],
            "xvT": xT[("v", b)],
            "wqT": np.ascontiguousarray(np.asarray(Wq, np.float32)[sl, :].T),
            "wkT": np.ascontiguousarray(np.asarray(Wk, np.float32)[sl, :].T),
            "wvT": np.ascontiguousarray(np.asarray(Wv, np.float32)[sl, :].T),
            "woT": np.ascontiguousarray(np.asarray(Wo, np.float32)[:, sl].T),
            "bq8": np.ascontiguousarray(np.asarray(bq, np.float32)[sl] / 8.0),
            "bk_": np.ascontiguousarray(np.asarray(bk, np.float32)[sl]),
        })
    return in_maps


def gather(results, bv, bo, Wo):
    const = (np.asarray(bo, np.float64)
             + np.asarray(bv, np.float64) @ np.asarray(Wo, np.float64).T)
    out = np.zeros((B, S, D_MODEL), np.float32)
    for c in range(8):
        out[c // 4] += results[c]["out"]
    out += const.astype(np.float32)
    return out


def kernel(q, k, v, Wq, bq, Wk, bk, Wv, bv, Wo, bo):
    nc = _get_nc()
    in_maps = make_in_maps(q, k, v, Wq, bq, Wk, bk, Wv, bv, Wo, bo)
    res = run_bass_kernel_spmd(nc, in_maps, list(range(8))).results
    return gather(res, bv, bo, Wo)
